# revision 1
# baseline (speedup 1.0000x reference)
"""Trainium2 Bass kernel for HarmonicDDSPEngine.

Strategy (T-sharding, 8 cores):
  - The (64, T) sin table is batch-independent and its values are pinned by
    fp32 argument quantization (args up to 7e5 rad), so it is precomputed on
    host bit-exactly the way the jax fp32 reference computes it, and each
    core receives only its T-chunk (5.6 MB) -- 8x less table traffic than
    batch sharding.
  - Each core owns a contiguous chunk of 22080 samples for ALL 16 batches.
    Layout on chip: partition p = b*8 + j (16 batches x 8 t-subblocks of
    L=2760), free dim n in [0, 2760). This keeps every elementwise op at
    full 128-partition width.
  - Harmonics einsum runs on TensorE as 4 PSUM-accumulated matmuls per
    output tile with block-diagonal weights (K=128 = 64 harmonics x 2
    subblocks), producing (128, 460) tiles directly in the (b,j) layout.
  - ADSR envelope (x gain) is evaluated exactly with an affine/relu/min
    lattice: env*g = relu(min(att, decS) - wu) with
      att  = relu(sc_att*n + bi_att)          (ACT)
      z    = relu(sc_z*n + bi_z)              (ACT)
      decS = sc_d2*z + bi_d2   (= max(dec, sus)*g)  (ACT)
      wu   = relu(sc_w*n + bi_w)  (= g*sus/(r-1)*relu(i - (a+d+s)))  (ACT)
    which reproduces the reference's index-based piecewise selects exactly
    at integer sample points (boundary crossings proven value-exact).
  - Global per-batch abs-max: local free-dim reduce -> cross-partition
    j-fold via a DRAM roundtrip row view -> 64-byte AllReduce(max) across
    the 8 cores -> broadcast back to partitions with a tiny selector matmul
    -> reciprocal -> one tensor_scalar multiply.
"""

import os
import numpy as np

import concourse.bass as bass
import concourse.bacc as bacc
import concourse.mybir as mybir
import concourse.tile as tile
from concourse.bass_utils import run_bass_kernel_spmd

F32 = mybir.dt.float32
F16 = mybir.dt.float16
f32 = np.float32

B, T, NH = 16, 176400, 64
SR = 44100
NCORES = 8
J = 8             # t-subblocks per core
L = 2760          # samples per subblock
TC = J * L        # 22080 per-core chunk
TPAD = NCORES * TC  # 176640
NT = 6            # PSUM tiles per core
N = L // NT       # 460, fits one PSUM bank
NQ = 4            # K=128 block-diagonal accumulation steps
NS2 = 3           # rhs DMA chunks of 2 tiles (920 cols) each

_cache = {}


def _build_nc():
    nc = bacc.Bacc(None, num_devices=NCORES)

    rhs_d = nc.dram_tensor("rhs_t", [NQ, 128, L], F16, kind="ExternalInput")
    lhs_d = nc.dram_tensor("lhsT", [NQ, 128, 128], F16, kind="ExternalInput")
    noise_d = nc.dram_tensor("noise_p", [128, L], F32, kind="ExternalInput")
    consts_d = nc.dram_tensor("consts", [128, 16], F32, kind="ExternalInput")
    sel_d = nc.dram_tensor("sel16", [16, 128], F32, kind="ExternalInput")
    out_d = nc.dram_tensor("out_sig", [128, L], F32, kind="ExternalOutput")

    mx_d = nc.dram_tensor("mx_stage", [128, 1], F32)
    cc_in = nc.dram_tensor("cc_in", [16, 1], F32)
    cc_out = nc.dram_tensor("cc_out", [8, 16], F32, addr_space="Shared")

    AF = mybir.ActivationFunctionType
    OP = mybir.AluOpType

    with tile.TileContext(nc) as tc:
        with (
            tc.tile_pool(name="const", bufs=1) as cpool,
            tc.tile_pool(name="rhs", bufs=NQ * NS2) as rpool,
            tc.tile_pool(name="env", bufs=12) as epool,
            tc.tile_pool(name="sig", bufs=1) as spool,
            tc.tile_pool(name="small", bufs=8) as smpool,
            tc.tile_pool(name="psum", bufs=NT, space="PSUM") as ppool,
            tc.tile_pool(name="psb", bufs=1, space="PSUM") as pbpool,
            tc.tile_pool(name="psx", bufs=1, space="PSUM") as pxpool,
        ):
            consts = cpool.tile([128, 16], F32, tag="consts")
            nc.gpsimd.dma_start(consts[:], consts_d[:])
            lhs = cpool.tile([128, NQ, 128], F16, tag="lhs")
            nc.gpsimd.dma_start(lhs[:], lhs_d[:].rearrange("q p m -> p q m"))
            sel16 = cpool.tile([16, 128], F32, tag="sel16")
            nc.gpsimd.dma_start(sel16[:], sel_d[:])
            noise_t = cpool.tile([128, L], F32, tag="noise_t")
            nc.sync.dma_start(noise_t[:], noise_d[:])
            # iota row 0..L-1 (fp32-exact), same on every partition
            iot = cpool.tile([128, L], F32, tag="iot")
            nc.gpsimd.iota(iot[:], [[1, L]], base=0, channel_multiplier=0,
                           allow_small_or_imprecise_dtypes=True)

            def cst(i):
                return consts[:, i:i + 1]

            # noise affine on DVE: n = noise*(2*lev) - lev (per-partition)
            nsig = cpool.tile([128, L], F32, tag="nsig")
            nc.vector.tensor_scalar(nsig[:], noise_t[:], cst(8), cst(9),
                                    OP.mult, OP.add)
            # ---- envelope * gain, exact lattice, in two pipelined halves ----
            H2 = L // 2
            envgs = []
            for h in range(2):
                hl = slice(h * H2, (h + 1) * H2)
                att = epool.tile([128, H2], F32, tag="env", name=f"att{h}")
                nc.scalar.activation(att[:], iot[:, hl], AF.Relu,
                                     bias=cst(1), scale=cst(0))
                z = epool.tile([128, H2], F32, tag="env", name=f"z{h}")
                nc.scalar.activation(z[:], iot[:, hl], AF.Relu,
                                     bias=cst(3), scale=cst(2))
                decs = epool.tile([128, H2], F32, tag="env", name=f"decs{h}")
                nc.scalar.activation(decs[:], z[:], AF.Identity,
                                     bias=cst(5), scale=cst(4))
                wu = epool.tile([128, H2], F32, tag="env", name=f"wu{h}")
                nc.scalar.activation(wu[:], iot[:, hl], AF.Relu,
                                     bias=cst(7), scale=cst(6))
                mm = epool.tile([128, H2], F32, tag="env", name=f"mm{h}")
                nc.vector.tensor_tensor(mm[:], att[:], decs[:], OP.min)
                env0 = epool.tile([128, H2], F32, tag="env", name=f"env0{h}")
                nc.vector.tensor_tensor(env0[:], mm[:], wu[:], OP.subtract)
                envg = cpool.tile([128, H2], F32, tag=f"envg{h}",
                                  name=f"envg{h}")
                nc.scalar.activation(envg[:], env0[:], AF.Relu)
                envgs.append(envg)

            # ---- harmonics matmuls + signal chain, tiled ----
            sig = spool.tile([128, L], F32, tag="sig")
            mxcols = smpool.tile([128, NT], F32, tag="mxc")

            psums = [ppool.tile([128, N], F32, tag="ps", name=f"ps{i}")
                     for i in range(NT)]
            chunks = {}
            for s2 in range(NS2):
                for q in range(NQ):
                    ch = rpool.tile([128, 2 * N], F16, tag="rhs",
                                    name=f"rhs{q}_{s2}")
                    nc.sync.dma_start(
                        ch[:], rhs_d[q, :, s2 * 2 * N:(s2 + 1) * 2 * N])
                    chunks[(q, s2)] = ch

            # tiny absorber matmuls: pull the lhs/sel16 DMA waits onto PE so
            # real matmuls and the broadcast matmul each need only one wait
            scr = pbpool.tile([128, 1], F32, tag="scr", name="scr")
            nc.tensor.matmul(scr[:], lhs[:, 0, :], lhs[:, 0, 0:1],
                             start=True, stop=True)
            nc.tensor.matmul(scr[:], sel16[:], sel16[:, 0:1],
                             start=True, stop=True)

            for s in range(NT):
                ps = psums[s]
                s2, half = divmod(s, 2)
                for q in range(NQ):
                    ch = chunks[(q, s2)]
                    nc.tensor.matmul(
                        ps[:], lhs[:, q, :], ch[:, half * N:(half + 1) * N],
                        start=(q == 0), stop=(q == NQ - 1))
                sl = slice(s * N, (s + 1) * N)
                # h + n
                nc.vector.tensor_tensor(sig[:, sl], ps[:], nsig[:, sl], OP.add)
                # * (env*gain)
                eh = envgs[s // 3]
                el = slice((s % 3) * N, (s % 3 + 1) * N)
                nc.vector.tensor_tensor(sig[:, sl], sig[:, sl], eh[:, el],
                                        OP.mult)
                # per-tile abs-max
                nc.vector.tensor_reduce(mxcols[:, s:s + 1], sig[:, sl],
                                        axis=mybir.AxisListType.X, op=OP.max,
                                        apply_absolute_value=True)

            # ---- global max: fold tiles, fold j across partitions, allreduce
            mx = smpool.tile([128, 1], F32, tag="mx")
            nc.vector.tensor_reduce(mx[:], mxcols[:], axis=mybir.AxisListType.X,
                                    op=OP.max)
            nc.sync.dma_start(mx_d[:], mx[:])
            row = smpool.tile([1, 128], F32, tag="row")
            nc.sync.dma_start(row[:], mx_d[:].rearrange("p o -> o p"))
            row16 = smpool.tile([1, 16], F32, tag="row16")
            nc.vector.tensor_reduce(row16[:],
                                    row[:].rearrange("o (b j) -> o b j", j=J),
                                    axis=mybir.AxisListType.X, op=OP.max)
            nc.vector.tensor_scalar(row16[:], row16[:], 1e-5, None, OP.add)
            nc.sync.dma_start(cc_in[:].rearrange("b o -> o b"), row16[:])
            # AllGather (no reduce penalty) + local 8-way max per batch
            nc.gpsimd.collective_compute(
                "AllGather", OP.bypass,
                replica_groups=[list(range(NCORES))],
                ins=[cc_in[:]], outs=[cc_out[:]])
            col8 = smpool.tile([16, 8], F32, tag="col8")
            nc.sync.dma_start(col8[:], cc_out[:].rearrange("c b -> b c"))
            col16 = smpool.tile([16, 1], F32, tag="col16")
            nc.vector.tensor_reduce(col16[:], col8[:],
                                    axis=mybir.AxisListType.X, op=OP.max)
            # broadcast to (128,1) via selector matmul, then reciprocal
            bmax = pxpool.tile([128, 1], F32, tag="bmax")
            nc.tensor.matmul(bmax[:], sel16[:], col16[:], start=True, stop=True)
            inv = smpool.tile([128, 1], F32, tag="inv")
            nc.vector.reciprocal(inv[:], bmax[:])

            # ---- normalize + store (split so the DMA overlaps the div) ----
            H = L // 2
            nc.vector.tensor_scalar(sig[:, 0:H], sig[:, 0:H], inv[:], None,
                                    OP.mult)
            nc.sync.dma_start(out_d[:, 0:H], sig[:, 0:H])
            nc.vector.tensor_scalar(sig[:, H:L], sig[:, H:L], inv[:], None,
                                    OP.mult)
            nc.sync.dma_start(out_d[:, H:L], sig[:, H:L])

    nc.finalize()
    return nc


def _host_prep(harmonic_dist, noise_bands, adsr, gain, noise):
    """All fp32 ops replicate the jax-CPU reference bit-exactly where it
    matters (sin-table arguments); envelope constants are f64->f32."""
    # sin table, bit-exact args
    step = f32(np.float64(T / SR) / (T - 1))
    t = np.arange(TPAD, dtype=f32) * step
    k = np.arange(1, NH + 1, dtype=f32)
    ck = f32(2.0 * np.pi * 440.0) * k
    S = np.zeros((NH, TPAD), f32)
    np.sin(ck[:, None] * t[None, :T], out=S[:, :T])

    A = np.ascontiguousarray(harmonic_dist, dtype=f32)
    lhsT = np.zeros((NQ, 128, 128), f32)
    for q in range(NQ):
        for jl in range(2):
            jj = 2 * q + jl
            lhsT[q, jl * 64:(jl + 1) * 64, :].reshape(64, 16, 8)[:, :, jj] = A.T

    lhsT16 = lhsT.astype(np.float16)
    sel16 = np.zeros((16, 128), f32)
    sel16[np.arange(128) // 8, np.arange(128)] = 1.0

    npad = np.full((B, TPAD), 0.5, f32)
    npad[:, :T] = noise

    # ADSR int constants, replicating reference rounding exactly
    att_in, dec_in, sus, rel_in = (adsr[:, 0].astype(f32), adsr[:, 1].astype(f32),
                                   adsr[:, 2].astype(f32), adsr[:, 3].astype(f32))
    a = np.floor((att_in * f32(0.5)) * f32(SR)).astype(np.int64) + 1
    d = np.floor((dec_in * f32(0.5)) * f32(SR)).astype(np.int64) + 1
    r = np.floor((rel_in * f32(0.5)) * f32(SR)).astype(np.int64) + 1
    total = a + d + r
    scale = (f32(T) / total.astype(f32)).astype(f32)
    resc = total > T
    a = np.where(resc, np.floor(a.astype(f32) * scale).astype(np.int64), a)
    d = np.where(resc, np.floor(d.astype(f32) * scale).astype(np.int64), d)
    r = np.where(resc, np.floor(r.astype(f32) * scale).astype(np.int64), r)
    s = np.maximum(T - (a + d + r), 0)

    g64 = gain.astype(np.float64)[:, 0]
    sus64 = sus.astype(np.float64)
    m_a = np.maximum(a - 1, 1).astype(np.float64)
    m_d = np.maximum(d - 1, 1).astype(np.float64)
    m_r = np.maximum(r - 1, 1).astype(np.float64)
    A2 = (a + d + s).astype(np.float64)
    lev64 = (np.mean(noise_bands.astype(f32), axis=1, dtype=f32)
             * f32(0.1)).astype(np.float64)

    in_maps = []
    for c in range(NCORES):
        rhs_c = np.ascontiguousarray(
            S[:, c * TC:(c + 1) * TC].reshape(NH, J, L)
            .transpose(1, 0, 2).reshape(NQ, 128, L).astype(np.float16))
        noise_c = np.ascontiguousarray(
            npad[:, c * TC:(c + 1) * TC].reshape(128, L))

        consts = np.zeros((128, 16), np.float64)
        for b in range(B):
            for j in range(J):
                p = b * 8 + j
                base = c * TC + j * L
                sc_att = g64[b] / m_a[b]
                consts[p, 0] = sc_att
                consts[p, 1] = f32(sc_att) * np.float64(base)
                sc_z = -1.0 / m_d[b]
                consts[p, 2] = sc_z
                consts[p, 3] = 1.0 - (base - a[b]) / m_d[b]
                consts[p, 4] = (1.0 - sus64[b]) * g64[b]
                consts[p, 5] = sus64[b] * g64[b]
                sc_w = sus64[b] * g64[b] / m_r[b]
                consts[p, 6] = sc_w
                consts[p, 7] = -f32(sc_w) * (A2[b] - np.float64(base))
                consts[p, 8] = 2.0 * lev64[b]
                consts[p, 9] = -lev64[b]
        in_maps.append({
            "rhs_t": rhs_c,
            "lhsT": lhsT16,
            "noise_p": noise_c,
            "consts": consts.astype(f32),
            "sel16": sel16,
        })
    return in_maps


LAST_RESULTS = None


def kernel(base_audio, harmonic_dist, noise_bands, adsr, gain, noise):
    global LAST_RESULTS
    if "nc" not in _cache:
        _cache["nc"] = _build_nc()
    nc = _cache["nc"]

    in_maps = _host_prep(
        np.asarray(harmonic_dist), np.asarray(noise_bands),
        np.asarray(adsr), np.asarray(gain), np.asarray(noise))

    trace = bool(os.environ.get("KERNEL_TRACE"))
    res = run_bass_kernel_spmd(nc, in_maps, list(range(NCORES)), trace=trace)
    LAST_RESULTS = res

    out = np.empty((B, TPAD), f32)
    for c in range(NCORES):
        out[:, c * TC:(c + 1) * TC] = res.results[c]["out_sig"].reshape(B, TC)
    return np.ascontiguousarray(out[:, :T])

